# revision 6
# baseline (speedup 1.0000x reference)
"""CoAttenBlock Trainium2 kernel.

Full inputs in, full outputs out. Shards batch (B=8) across 8 NeuronCores,
one sample per core (pure data parallel, no collectives).

Per-core math (C=64, HW=2304, strips of 128 along the left position axis n):
  XL = WL @ [xlh;xll] + bL                      [64, 2304]
  XR = WR @ [xrh;xrl] + bR
  per strip s: aff_s = XL[:,s].T @ XR           [128, 2304]
               E_s   = exp(aff_s), rowsum via activation accum_out
               r2    = 1/rowsum  (folded into the strip's YRT weight columns)
               P12  += [YLT_s | YRT_s*r2].T @ E_s   (PSUM-resident [128, 2304])
  where YLT = (WLo_r @ XL).T strips, YRT = (WRo_r @ XR).T strips, so that
  P1 = WLo_r @ (XL @ E) and P2 = WRo_r @ (XR @ diag(r2) @ E).
  Gate pre-acts are recovered as vL.T @ P1 with vL = solve(WLo_r.T, gwL.T)
  (host-side 64x64 solve; inputs are deterministic so conditioning is checked
  implicitly by the accuracy test).
  colsum = ones.T @ (sum of E_s)  (two SBUF accumulators: DVE + GPSIMD chains)
  s1 = sigmoid(g1pre * r1 + gb1) * r1,  r1 = 1/colsum
  out_L = WLo_l @ XL + P1 * s1[m] + bLo ; out_R analogous with s2 = sigmoid(.)
"""

import os
import sys

import numpy as np

if os.path.isdir("/opt/trn_rl_repo") and "/opt/trn_rl_repo" not in sys.path:
    sys.path.insert(0, "/opt/trn_rl_repo")

import concourse.bass as bass
import concourse.tile as tile
from concourse import bacc, mybir
from concourse.bass_utils import run_bass_kernel_spmd

B, C, H, W = 8, 64, 48, 48
HW = H * W            # 2304
C2 = 2 * C            # 128
NSTRIP = HW // 128    # 18
F32 = mybir.dt.float32
F32R = mybir.dt.float32r
AF = mybir.ActivationFunctionType

# m-chunking helpers: matmul moving-dim max is 512 fp32 and each matmul
# output must sit inside one 2KB PSUM bank.
def chunks(total, step=512):
    out = []
    c0 = 0
    while c0 < total:
        out.append((c0, min(step, total - c0)))
        c0 += step
    return out


CH_2304 = chunks(2304)            # 4x512 + 256
CH_1152 = chunks(1152)            # 2x512 + 128
HALF = 1152


def r(ap):
    return ap.bitcast(F32R)


def build_nc():
    nc = bacc.Bacc("TRN2", target_bir_lowering=False, debug=False)

    x2l_d = nc.dram_tensor("x2l", [C2, HW], F32, kind="ExternalInput").ap()
    x2r_d = nc.dram_tensor("x2r", [C2, HW], F32, kind="ExternalInput").ap()
    wlT_d = nc.dram_tensor("wlT", [C2, C], F32, kind="ExternalInput").ap()
    wrT_d = nc.dram_tensor("wrT", [C2, C], F32, kind="ExternalInput").ap()
    wloRT_d = nc.dram_tensor("wloRT", [C, C], F32, kind="ExternalInput").ap()
    wroRT_d = nc.dram_tensor("wroRT", [C, C], F32, kind="ExternalInput").ap()
    wloLT_d = nc.dram_tensor("wloLT", [C, C], F32, kind="ExternalInput").ap()
    wroLT_d = nc.dram_tensor("wroLT", [C, C], F32, kind="ExternalInput").ap()
    vL_d = nc.dram_tensor("vL", [C, 1], F32, kind="ExternalInput").ap()
    vR_d = nc.dram_tensor("vR", [C, 1], F32, kind="ExternalInput").ap()
    bL_d = nc.dram_tensor("bL", [C, 1], F32, kind="ExternalInput").ap()
    bR_d = nc.dram_tensor("bR", [C, 1], F32, kind="ExternalInput").ap()
    bLo_d = nc.dram_tensor("bLo", [C, 1], F32, kind="ExternalInput").ap()
    bRo_d = nc.dram_tensor("bRo", [C, 1], F32, kind="ExternalInput").ap()
    gb_d = nc.dram_tensor("gb", [1, 2], F32, kind="ExternalInput").ap()
    id64_d = nc.inline_tensor(np.eye(C, dtype=np.float32), "id64").ap()

    out_l_d = nc.dram_tensor("out_l", [C, HW], F32, kind="ExternalOutput").ap()
    out_r_d = nc.dram_tensor("out_r", [C, HW], F32, kind="ExternalOutput").ap()

    with tile.TileContext(nc) as tc:
        import contextlib

        with contextlib.ExitStack() as outer:
            consts = outer.enter_context(tc.tile_pool(name="consts", bufs=1))
            big = outer.enter_context(tc.tile_pool(name="big", bufs=1))
            epool = outer.enter_context(tc.tile_pool(name="epool", bufs=2))
            smalls = outer.enter_context(tc.tile_pool(name="smalls", bufs=3))
            ph3sb = outer.enter_context(tc.tile_pool(name="ph3sb", bufs=2))

            # ---- constants / weights to SBUF ----
            wlT = consts.tile([C2, C], F32)
            wrT = consts.tile([C2, C], F32)
            wloRT_raw = consts.tile([C, C], F32)
            wroRT_raw = consts.tile([C, C], F32)
            wloLT_raw = consts.tile([C, C], F32)
            wroLT_raw = consts.tile([C, C], F32)
            vL_raw = consts.tile([C, 1], F32)
            vR_raw = consts.tile([C, 1], F32)
            wloRT = consts.tile([C, C], F32)
            wroRT = consts.tile([C, C], F32)
            wloLT = consts.tile([C, C], F32)
            wroLT = consts.tile([C, C], F32)
            vL = consts.tile([C, 1], F32)
            vR = consts.tile([C, 1], F32)
            bL = consts.tile([C, 1], F32)
            bR = consts.tile([C, 1], F32)
            bLo = consts.tile([C, 1], F32)
            bRo = consts.tile([C, 1], F32)
            gb = consts.tile([1, 2], F32)
            id64_raw = consts.tile([C, C], F32)
            id64 = consts.tile([C, C], F32)
            ones128_raw = consts.tile([C2, 1], F32)
            ones64_raw = consts.tile([1, C], F32)
            ones128 = consts.tile([C2, 1], F32)
            ones64 = consts.tile([1, C], F32)
            for dst, src in [
                (wlT, wlT_d), (wrT, wrT_d),
                (wloRT_raw, wloRT_d), (wroRT_raw, wroRT_d),
                (wloLT_raw, wloLT_d), (wroLT_raw, wroLT_d),
                (vL_raw, vL_d), (vR_raw, vR_d),
                (bL, bL_d), (bR, bR_d), (bLo, bLo_d), (bRo, bRo_d),
                (gb, gb_d), (id64_raw, id64_d),
            ]:
                nc.gpsimd.dma_start(out=dst, in_=src)
            nc.vector.memset(ones128_raw, 1.0)
            nc.vector.memset(ones64_raw, 1.0)
            for dst, srcc in [(ones128, ones128_raw), (ones64, ones64_raw),
                              (wloRT, wloRT_raw), (wroRT, wroRT_raw),
                              (wloLT, wloLT_raw), (wroLT, wroLT_raw),
                              (vL, vL_raw), (vR, vR_raw), (id64, id64_raw)]:
                nc.scalar.copy(r(dst), srcc)

            # ---- big SBUF tensors ----
            x2l = big.tile([C2, HW], F32)
            x2r = big.tile([C2, HW], F32)
            XL = big.tile([C, HW], F32)
            XR = big.tile([C, HW], F32)
            Wc = big.tile([C2, HW], F32)       # 18 strips of [YLT | YRT]
            csum_a = big.tile([C2, HW], F32)   # DVE-chain E accumulator
            csum_b = big.tile([C2, HW], F32)   # GPSIMD-chain E accumulator
            P1sb = big.tile([C, HW], F32)
            P2sb = big.tile([C, HW], F32)
            outLR = big.tile([C2, HW], F32)
            outL = outLR[0:C, :]
            outR = outLR[C:C2, :]

            nc.sync.dma_start(out=x2l, in_=x2l_d)
            nc.sync.dma_start(out=x2r, in_=x2r_d)

            with contextlib.ExitStack() as ph2_psum:
                p12p = ph2_psum.enter_context(
                    tc.tile_pool(name="p12p", bufs=1, space="PSUM"))
                affp = ph2_psum.enter_context(
                    tc.tile_pool(name="affp", bufs=1, space="PSUM"))
                P12 = p12p.tile([C2, HW], F32)  # 5 banks, lives all of phase 1+2

                # ---- phase 1: convs (full fp32) ----
                # XL/XR via P12 scratch regions; partitions 0:64 for L, 64:128 for R
                for c0, cn in CH_2304:
                    nc.tensor.matmul(P12[0:C, c0:c0 + cn], wlT,
                                     x2l[:, c0:c0 + cn], start=True, stop=True)
                    nc.scalar.activation(r(XL[:, c0:c0 + cn]),
                                         P12[0:C, c0:c0 + cn],
                                         AF.Identity, bias=bL, scale=1.0)
                    nc.tensor.matmul(P12[C:C2, c0:c0 + cn], wrT,
                                     x2r[:, c0:c0 + cn], start=True, stop=True)
                    nc.scalar.activation(r(XR[:, c0:c0 + cn]),
                                         P12[C:C2, c0:c0 + cn],
                                         AF.Identity, bias=bR, scale=1.0)

                # YLT/YRT strips -> Wc (full fp32), via P12 64-col scratch slices
                for s in range(NSTRIP):
                    ysl = slice(64 * s, 64 * s + 64)
                    nc.tensor.matmul(P12[:, ysl], r(XL[:, 128 * s:128 * s + 128]),
                                     r(wloRT), start=True, stop=True)
                    nc.vector.tensor_copy(r(Wc[:, 128 * s:128 * s + 64]),
                                          P12[:, ysl])
                for s in range(NSTRIP):
                    ysl = slice(64 * (NSTRIP + s), 64 * (NSTRIP + s) + 64)
                    nc.tensor.matmul(P12[:, ysl], r(XR[:, 128 * s:128 * s + 128]),
                                     r(wroRT), start=True, stop=True)
                    nc.vector.tensor_copy(r(Wc[:, 128 * s + 64:128 * s + 128]),
                                          P12[:, ysl])

                # ---- phase 2: strip loop ----
                for s in range(NSTRIP):
                    E = epool.tile([C2, HW], F32, tag="e", name=f"E_{s}")
                    rs = smalls.tile([C2, 2], F32, tag="rs", name=f"rs_{s}")
                    lhs_aff = r(XL[:, 128 * s:128 * s + 128])
                    for h in range(2):
                        AFF = affp.tile([C2, HALF], F32, tag="aff",
                                        name=f"aff_{s}_{h}")
                        for c0, cn in CH_1152:
                            nc.tensor.matmul(
                                AFF[:, c0:c0 + cn], lhs_aff,
                                r(XR[:, HALF * h + c0:HALF * h + c0 + cn]),
                                start=True, stop=True)
                        nc.scalar.activation(
                            r(E[:, HALF * h:HALF * h + HALF]), AFF, AF.Exp,
                            accum_out=rs[:, h:h + 1])
                    rowsum = smalls.tile([C2, 1], F32, tag="rowsum",
                                         name=f"rowsum_{s}")
                    r2 = smalls.tile([C2, 1], F32, tag="r2", name=f"r2_{s}")
                    nc.vector.tensor_add(rowsum, rs[:, 0:1], rs[:, 1:2])
                    nc.vector.reciprocal(r2, rowsum)
                    wright = Wc[:, 128 * s + 64:128 * s + 128]
                    nc.vector.tensor_scalar_mul(r(wright), wright, r2)
                    lhs_b = r(Wc[:, 128 * s:128 * s + 128])
                    for c0, cn in CH_2304:
                        nc.tensor.matmul(P12[:, c0:c0 + cn], lhs_b,
                                         r(E[:, c0:c0 + cn]),
                                         start=(s == 0), stop=(s == NSTRIP - 1))
                    # colsum partial accumulation: two independent chains
                    if s == 0:
                        nc.vector.tensor_copy(r(csum_a), E)
                    elif s == 1:
                        nc.gpsimd.tensor_copy(r(csum_b), E)
                    elif s % 8 < 5:
                        nc.vector.tensor_add(r(csum_a), csum_a, E)
                    else:
                        nc.gpsimd.tensor_add(r(csum_b), csum_b, E)

                # drain P12
                nc.vector.tensor_copy(r(P1sb), P12[0:C, :])
                nc.scalar.copy(r(P2sb), P12[C:C2, :])

            # phase-2 PSUM pools released; phase 3 uses half-width tiles
            with tc.tile_pool(name="ph3p", bufs=2, space="PSUM") as ph3:
                for h in range(2):
                    msl = slice(HALF * h, HALF * h + HALF)

                    cs = ph3.tile([1, HALF], F32, tag="ph3", name=f"cs_{h}")
                    for c0, cn in CH_1152:
                        nc.tensor.matmul(cs[:, c0:c0 + cn], r(ones128),
                                         r(csum_a[:, HALF * h + c0:HALF * h + c0 + cn]),
                                         start=True, stop=False)
                        nc.tensor.matmul(cs[:, c0:c0 + cn], r(ones128),
                                         r(csum_b[:, HALF * h + c0:HALF * h + c0 + cn]),
                                         start=False, stop=True)
                    r1 = ph3sb.tile([1, HALF], F32, tag="r1", name=f"r1_{h}")
                    nc.vector.reciprocal(r1, cs)

                    g1p = ph3.tile([1, HALF], F32, tag="ph3", name=f"g1p_{h}")
                    for c0, cn in CH_1152:
                        nc.tensor.matmul(g1p[:, c0:c0 + cn], r(vL),
                                         r(P1sb[:, HALF * h + c0:HALF * h + c0 + cn]),
                                         start=True, stop=True)
                    g1pre = ph3sb.tile([1, HALF], F32, tag="g1pre",
                                        name=f"g1pre_{h}")
                    nc.vector.tensor_mul(g1pre, g1p, r1)
                    g1 = ph3sb.tile([1, HALF], F32, tag="g1", name=f"g1_{h}")
                    nc.scalar.activation(g1, g1pre, AF.Sigmoid,
                                         bias=gb[0:1, 0:1], scale=1.0)
                    s1 = ph3sb.tile([1, HALF], F32, tag="s1", name=f"s1_{h}")
                    nc.vector.tensor_mul(r(s1), g1, r1)

                    g2p = ph3.tile([1, HALF], F32, tag="ph3", name=f"g2p_{h}")
                    for c0, cn in CH_1152:
                        nc.tensor.matmul(g2p[:, c0:c0 + cn], r(vR),
                                         r(P2sb[:, HALF * h + c0:HALF * h + c0 + cn]),
                                         start=True, stop=True)
                    g2 = ph3sb.tile([1, HALF], F32, tag="g2", name=f"g2_{h}")
                    nc.scalar.activation(r(g2), g2p, AF.Sigmoid,
                                         bias=gb[0:1, 1:2], scale=1.0)

                    S1 = ph3.tile([C, HALF], F32, tag="ph3", name=f"S1_{h}")
                    for c0, cn in CH_1152:
                        nc.tensor.matmul(S1[:, c0:c0 + cn], r(ones64),
                                         r(s1[:, c0:c0 + cn]),
                                         start=True, stop=True)
                    t1 = ph3sb.tile([C, HALF], F32, tag="t1", name=f"t1_{h}")
                    nc.vector.tensor_mul(r(t1), P1sb[:, msl], S1)

                    S2 = ph3.tile([C, HALF], F32, tag="ph3", name=f"S2_{h}")
                    for c0, cn in CH_1152:
                        nc.tensor.matmul(S2[:, c0:c0 + cn], r(ones64),
                                         r(g2[:, c0:c0 + cn]),
                                         start=True, stop=True)
                    t2 = ph3sb.tile([C, HALF], F32, tag="t2", name=f"t2_{h}")
                    nc.vector.tensor_mul(r(t2), P2sb[:, msl], S2)

                    # out_L = WLo_l @ XL + t1 (+bLo on the way out)
                    OL = ph3.tile([C, HALF], F32, tag="ph3", name=f"OL_{h}")
                    for c0, cn in CH_1152:
                        nc.tensor.matmul(OL[:, c0:c0 + cn], r(wloLT),
                                         r(XL[:, HALF * h + c0:HALF * h + c0 + cn]),
                                         start=True, stop=False)
                        nc.tensor.matmul(OL[:, c0:c0 + cn], r(id64),
                                         r(t1[:, c0:c0 + cn]),
                                         start=False, stop=True)
                    nc.scalar.activation(outL[:, msl], OL, AF.Identity,
                                         bias=bLo, scale=1.0)

                    OR_ = ph3.tile([C, HALF], F32, tag="ph3", name=f"OR_{h}")
                    for c0, cn in CH_1152:
                        nc.tensor.matmul(OR_[:, c0:c0 + cn], r(wroLT),
                                         r(XR[:, HALF * h + c0:HALF * h + c0 + cn]),
                                         start=True, stop=False)
                        nc.tensor.matmul(OR_[:, c0:c0 + cn], r(id64),
                                         r(t2[:, c0:c0 + cn]),
                                         start=False, stop=True)
                    nc.scalar.activation(outR[:, msl], OR_, AF.Identity,
                                         bias=bRo, scale=1.0)

            nc.sync.dma_start(out=out_l_d, in_=outL)
            nc.sync.dma_start(out=out_r_d, in_=outR)

    nc.compile()
    return nc


_NC_CACHE = {}


def _get_nc():
    if "nc" not in _NC_CACHE:
        _NC_CACHE["nc"] = build_nc()
    return _NC_CACHE["nc"]


def _prep_shared(concaL_w, concaL_b, concaR_w, concaR_b,
                 gateL_w, gateL_b, gateR_w, gateR_b,
                 concaLo_w, concaLo_b, concaRo_w, concaRo_b):
    f = np.float32
    wloR = np.asarray(concaLo_w)[:, C:].astype(np.float64)
    wroR = np.asarray(concaRo_w)[:, C:].astype(np.float64)
    vL = np.linalg.solve(wloR.T, np.asarray(gateL_w).astype(np.float64).reshape(C))
    vR = np.linalg.solve(wroR.T, np.asarray(gateR_w).astype(np.float64).reshape(C))
    return {
        "wlT": np.ascontiguousarray(np.asarray(concaL_w).T, dtype=f),
        "wrT": np.ascontiguousarray(np.asarray(concaR_w).T, dtype=f),
        "wloRT": np.ascontiguousarray(wloR.T, dtype=f),
        "wroRT": np.ascontiguousarray(wroR.T, dtype=f),
        "wloLT": np.ascontiguousarray(np.asarray(concaLo_w)[:, :C].T, dtype=f),
        "wroLT": np.ascontiguousarray(np.asarray(concaRo_w)[:, :C].T, dtype=f),
        "vL": np.ascontiguousarray(vL.reshape(C, 1), dtype=f),
        "vR": np.ascontiguousarray(vR.reshape(C, 1), dtype=f),
        "bL": np.ascontiguousarray(np.asarray(concaL_b).reshape(C, 1), dtype=f),
        "bR": np.ascontiguousarray(np.asarray(concaR_b).reshape(C, 1), dtype=f),
        "bLo": np.ascontiguousarray(np.asarray(concaLo_b).reshape(C, 1), dtype=f),
        "bRo": np.ascontiguousarray(np.asarray(concaRo_b).reshape(C, 1), dtype=f),
        "gb": np.array([[np.asarray(gateL_b).reshape(()),
                         np.asarray(gateR_b).reshape(())]], dtype=f),
    }


def kernel(xlh, xll, xrh, xrl,
           concaL_w, concaL_b, concaR_w, concaR_b,
           gateL_w, gateL_b, gateR_w, gateR_b,
           concaLo_w, concaLo_b, concaRo_w, concaRo_b,
           _return_results=False):
    nc = _get_nc()
    shared = _prep_shared(concaL_w, concaL_b, concaR_w, concaR_b,
                          gateL_w, gateL_b, gateR_w, gateR_b,
                          concaLo_w, concaLo_b, concaRo_w, concaRo_b)
    xlh = np.asarray(xlh, dtype=np.float32)
    xll = np.asarray(xll, dtype=np.float32)
    xrh = np.asarray(xrh, dtype=np.float32)
    xrl = np.asarray(xrl, dtype=np.float32)

    in_maps = []
    for c in range(B):
        x2l = np.concatenate([xlh[c].reshape(C, HW), xll[c].reshape(C, HW)], axis=0)
        x2r = np.concatenate([xrh[c].reshape(C, HW), xrl[c].reshape(C, HW)], axis=0)
        m = dict(shared)
        m["x2l"] = np.ascontiguousarray(x2l)
        m["x2r"] = np.ascontiguousarray(x2r)
        in_maps.append(m)

    res = run_bass_kernel_spmd(nc, in_maps, list(range(B)))
    out_L = np.stack([res.results[c]["out_l"].reshape(C, H, W) for c in range(B)])
    out_R = np.stack([res.results[c]["out_r"].reshape(C, H, W) for c in range(B)])
    if _return_results:
        return (out_L, out_R), res
    return (out_L, out_R)


# revision 20
# speedup vs baseline: 1.0211x; 1.0211x over previous
"""CoAttenBlock Trainium2 kernel.

Full inputs in, full outputs out. Shards batch (B=8) across 8 NeuronCores,
one sample per core (pure data parallel, no collectives).

Per-core math (C=64, HW=2304, strips of 128 along the left position axis n):
  XL = WL @ [xlh;xll] + bL                      [64, 2304]
  XR = WR @ [xrh;xrl] + bR
  per strip s: aff_s = XL[:,s].T @ XR           [128, 2304]
               E_s   = exp(aff_s), rowsum via activation accum_out
               r2    = 1/rowsum  (folded into the strip's YRT weight columns)
               P12  += [YLT_s | YRT_s*r2].T @ E_s   (PSUM-resident [128, 2304])
  where YLT = (WLo_r @ XL).T strips, YRT = (WRo_r @ XR).T strips, so that
  P1 = WLo_r @ (XL @ E) and P2 = WRo_r @ (XR @ diag(r2) @ E).
  Gate pre-acts are recovered as vL.T @ P1 with vL = solve(WLo_r.T, gwL.T)
  (host-side 64x64 solve; inputs are deterministic, conditioning ~5e2).
  colsum = ones.T @ (sum of E_s)  (two SBUF accumulators: DVE + GPSIMD chains)
  s1 = sigmoid(g1pre * r1 + gb1) * r1,  r1 = 1/colsum ; s2 = sigmoid(g2pre+gb2)
  out_L = WLo_l @ XL + P1 * s1[m] + bLo ; out_R = WRo_l @ XR + P2 * s2[m] + bRo

float32r (single-pass PE mode) is used for all large matmuls; producers of
f32r-matmul inputs write with a f32r-typed output AP so the engine rounds on
write (BIR verifier requirement). Convs and YLT/YRT stay higher precision.
"""

import os
import sys

import numpy as np

if os.path.isdir("/opt/trn_rl_repo") and "/opt/trn_rl_repo" not in sys.path:
    sys.path.insert(0, "/opt/trn_rl_repo")

import concourse.bass as bass
import concourse.tile as tile
from concourse import bacc, mybir
from concourse.bass_utils import run_bass_kernel_spmd

B, C, H, W = 8, 64, 48, 48
HW = H * W            # 2304
C2 = 2 * C            # 128
NSTRIP = HW // 128    # 18
F32 = mybir.dt.float32
F32R = mybir.dt.float32r
AF = mybir.ActivationFunctionType


def chunks(total, step=512):
    out = []
    c0 = 0
    while c0 < total:
        out.append((c0, min(step, total - c0)))
        c0 += step
    return out


CH_2304 = chunks(2304)            # 4x512 + 256


def r(ap):
    return ap.bitcast(F32R)


def build_nc():
    nc = bacc.Bacc("TRN2", target_bir_lowering=False, debug=False)

    x2l_d = nc.dram_tensor("x2l", [C2, HW], F32, kind="ExternalInput").ap()
    x2r_d = nc.dram_tensor("x2r", [C2, HW], F32, kind="ExternalInput").ap()
    wlT_d = nc.dram_tensor("wlT", [C2, C], F32, kind="ExternalInput").ap()
    wrT_d = nc.dram_tensor("wrT", [C2, C], F32, kind="ExternalInput").ap()
    wloRT_d = nc.dram_tensor("wloRT", [C, C], F32, kind="ExternalInput").ap()
    wroRT_d = nc.dram_tensor("wroRT", [C, C], F32, kind="ExternalInput").ap()
    wloLT_d = nc.dram_tensor("wloLT", [C, C], F32, kind="ExternalInput").ap()
    wroLT_d = nc.dram_tensor("wroLT", [C, C], F32, kind="ExternalInput").ap()
    vlr_d = nc.dram_tensor("vlr", [C2, 1], F32, kind="ExternalInput").ap()
    bL_d = nc.dram_tensor("bL", [C, 1], F32, kind="ExternalInput").ap()
    bR_d = nc.dram_tensor("bR", [C, 1], F32, kind="ExternalInput").ap()
    bLo_d = nc.dram_tensor("bLo", [C, 1], F32, kind="ExternalInput").ap()
    bRo_d = nc.dram_tensor("bRo", [C, 1], F32, kind="ExternalInput").ap()
    gb_d = nc.dram_tensor("gb", [1, 2], F32, kind="ExternalInput").ap()
    # identity stacked twice: rows 0:64 and 64:128 both hold I_64, so id-adds
    # can source either half of a [128, ...] tile at matching base partition
    id2_np = np.vstack([np.eye(C, dtype=np.float32), np.eye(C, dtype=np.float32)])
    id64b_d = nc.inline_tensor(id2_np, "id64b").ap()
    # selector for the merged S12 broadcast: out rows 0:64 <- s1, 64:128 <- g2
    sel_np = np.zeros((2, C2), dtype=np.float32)
    sel_np[0, 0:C] = 1.0
    sel_np[1, C:C2] = 1.0
    sel12_d = nc.inline_tensor(sel_np, "sel12").ap()

    out_l_d = nc.dram_tensor("out_l", [C, HW], F32, kind="ExternalOutput").ap()
    out_r_d = nc.dram_tensor("out_r", [C, HW], F32, kind="ExternalOutput").ap()

    with tile.TileContext(nc) as tc:
        import contextlib

        with contextlib.ExitStack() as outer:
            consts = outer.enter_context(tc.tile_pool(name="consts", bufs=1))
            big = outer.enter_context(tc.tile_pool(name="big", bufs=1))
            epool = outer.enter_context(tc.tile_pool(name="epool", bufs=3))
            smalls = outer.enter_context(tc.tile_pool(name="smalls", bufs=3))
            ph3sb = outer.enter_context(tc.tile_pool(name="ph3sb", bufs=2))

            # ---- constants / weights to SBUF ----
            wlT = consts.tile([C2, C], F32)
            wrT = consts.tile([C2, C], F32)
            wloRT_raw = consts.tile([C, C], F32)
            wroRT_raw = consts.tile([C, C], F32)
            wloLT_raw = consts.tile([C, C], F32)
            wroLT_raw = consts.tile([C, C], F32)
            vlr_raw = consts.tile([C2, 1], F32)
            id64b_raw = consts.tile([C2, C], F32)
            sel12_raw = consts.tile([2, C2], F32)
            ones128_raw = consts.tile([C2, 1], F32)
            ones64_raw = consts.tile([1, C], F32)
            wloRT = consts.tile([C, C], F32)
            wroRT = consts.tile([C, C], F32)
            wloLT = consts.tile([C, C], F32)
            wroLT = consts.tile([C, C], F32)
            vlr = consts.tile([C2, 1], F32)
            id64b = consts.tile([C2, C], F32)
            sel12 = consts.tile([2, C2], F32)
            ones128 = consts.tile([C2, 1], F32)
            ones64 = consts.tile([1, C], F32)
            bL = consts.tile([C, 1], F32)
            bR = consts.tile([C, 1], F32)
            bLo = consts.tile([C, 1], F32)
            bRo = consts.tile([C, 1], F32)
            gb = consts.tile([1, 2], F32)
            for dst, src in [
                (r(wlT), r(wlT_d)), (r(wrT), r(wrT_d)),
                (wloRT_raw, wloRT_d), (wroRT_raw, wroRT_d),
                (wloLT_raw, wloLT_d), (wroLT_raw, wroLT_d),
                (vlr_raw, vlr_d), (id64b_raw, id64b_d), (sel12_raw, sel12_d),
                (bL, bL_d), (bR, bR_d), (bLo, bLo_d), (bRo, bRo_d),
                (gb, gb_d),
            ]:
                nc.sync.dma_start(out=dst, in_=src)
            nc.vector.memset(ones128_raw, 1.0)
            nc.vector.memset(ones64_raw, 1.0)
            for dst, srcc in [(ones128, ones128_raw), (ones64, ones64_raw),
                              (wloRT, wloRT_raw), (wroRT, wroRT_raw),
                              (wloLT, wloLT_raw), (wroLT, wroLT_raw),
                              (vlr, vlr_raw), (id64b, id64b_raw),
                              (sel12, sel12_raw)]:
                nc.scalar.copy(r(dst), srcc)

            # ---- big SBUF tensors ----
            x2l = big.tile([C2, HW], F32)
            x2r = big.tile([C2, HW], F32)
            XL = big.tile([C, HW], F32)
            XR = big.tile([C, HW], F32)
            Wc = big.tile([C2, HW], F32)       # 18 strips of [YLT | YRT]
            csum_a = big.tile([C2, HW // 2], F32)  # DVE accumulates m[0:1152]
            csum_b = big.tile([C2, HW // 2], F32)  # Pool accumulates m[1152:]
            P12sb = big.tile([C2, HW], F32)    # drained P1 (0:64) / P2 (64:128)
            outLR = big.tile([C2, HW], F32)

            for c0, cn in CH_2304:
                nc.sync.dma_start(out=r(x2l[:, c0:c0 + cn]),
                                  in_=r(x2l_d[:, c0:c0 + cn]))
                nc.sync.dma_start(out=r(x2r[:, c0:c0 + cn]),
                                  in_=r(x2r_d[:, c0:c0 + cn]))

            with contextlib.ExitStack() as ph2_psum:
                p12p = ph2_psum.enter_context(
                    tc.tile_pool(name="p12p", bufs=1, space="PSUM"))
                affp = ph2_psum.enter_context(
                    tc.tile_pool(name="affp", bufs=1, space="PSUM"))
                P12 = p12p.tile([C2, HW], F32)  # 5 banks, lives all of phase 1+2
                ring = affp.tile([C2, 1536], F32, tag="ring", name="aff_ring")

                # ---- phase 1: convs (full fp32) + YLT/YRT build ----
                # After conv chunk j, emit the YLT/YRT strips of chunk j-1
                # (their XL/XR columns are copied by then); P12 is scratch.
                def emit_y(t):
                    ysl = slice(64 * t, 64 * t + 64)
                    nc.tensor.matmul(P12[:, ysl],
                                     r(XL[:, 128 * t:128 * t + 128]),
                                     r(wloRT), start=True, stop=True)
                    nc.vector.tensor_copy(r(Wc[:, 128 * t:128 * t + 64]),
                                          P12[:, ysl])
                    ysr = slice(64 * (NSTRIP + t), 64 * (NSTRIP + t) + 64)
                    nc.tensor.matmul(P12[:, ysr],
                                     r(XR[:, 128 * t:128 * t + 128]),
                                     r(wroRT), start=True, stop=True)
                    nc.vector.tensor_copy(r(Wc[:, 128 * t + 64:128 * t + 128]),
                                          P12[:, ysr])

                for j, (c0, cn) in enumerate(CH_2304):
                    nc.tensor.matmul(P12[0:C, c0:c0 + cn], r(wlT),
                                     r(x2l[:, c0:c0 + cn]), start=True, stop=True)
                    nc.scalar.activation(r(XL[:, c0:c0 + cn]),
                                         P12[0:C, c0:c0 + cn],
                                         AF.Identity, bias=bL, scale=1.0)
                    rsl = (j % 3) * 512
                    nc.tensor.matmul(ring[0:C, rsl:rsl + cn], r(wrT),
                                     r(x2r[:, c0:c0 + cn]), start=True, stop=True)
                    nc.vector.tensor_scalar_add(r(XR[:, c0:c0 + cn]),
                                                ring[0:C, rsl:rsl + cn], bR)
                    if j > 0:
                        for t in range(4 * (j - 1), 4 * j):
                            emit_y(t)
                for t in range(4 * (len(CH_2304) - 1), NSTRIP):
                    emit_y(t)

                # ---- phase 2: strip loop over a 3-slot aff ring ----
                # A_s = aff matmuls + merged exps + rowsum/recip for strip s.
                # B_s = Wc scale + bacc matmuls + colsum accumulate for s.
                # B lags A by 2 strips so PE always has bacc work to fill exp
                # waits; the YLT/YRT -> Wc build is emitted during the lag.
                phase = 0
                r2s = {}

                def emit_bacc(sb, c0, cn):
                    nc.tensor.matmul(P12[:, c0:c0 + cn],
                                     r(Wc[:, 128 * sb:128 * sb + 128]),
                                     r(Es[sb][:, c0:c0 + cn]),
                                     start=(sb == 0), stop=(sb == NSTRIP - 1))

                def emit_csum(sb):
                    E = Es[sb]
                    half = HW // 2
                    if sb == 0:
                        nc.vector.tensor_copy(r(csum_a), E[:, 0:half])
                        nc.gpsimd.tensor_copy(r(csum_b), E[:, half:HW])
                    else:
                        nc.vector.tensor_add(r(csum_a), csum_a, E[:, 0:half])
                        nc.gpsimd.tensor_add(r(csum_b), csum_b, E[:, half:HW])

                def emit_A(s, phase, sb):
                    # aff+exp for strip s, with strip sb's bacc matmuls
                    # interleaved between the aff pieces (PE is in-order; this
                    # keeps ACT fed with the next exp as early as possible).
                    if sb >= 0:
                        wright = Wc[:, 128 * sb + 64:128 * sb + 128]
                        nc.vector.tensor_scalar_mul(r(wright), wright, r2s[sb])
                    E = epool.tile([C2, HW], F32, tag="e", name=f"E_{s}")
                    rs = smalls.tile([C2, 4], F32, tag="rs", name=f"rs_{s}")
                    lhs_aff = r(XL[:, 128 * s:128 * s + 128])
                    pieces = [(p0, pn, (phase + i) % 3)
                              for i, (p0, pn) in enumerate(CH_2304)]
                    groups = []
                    for p0, pn, sl in pieces:
                        if groups and groups[-1][2] + groups[-1][1] == sl * 512 \
                                and groups[-1][1] + pn <= 1024:
                            groups[-1][1] += pn
                        else:
                            groups.append([p0, pn, sl * 512])
                    gidx = 0
                    done = 0
                    for i, (p0, pn, sl) in enumerate(pieces):
                        nc.tensor.matmul(ring[:, sl * 512:sl * 512 + pn],
                                         lhs_aff, r(XR[:, p0:p0 + pn]),
                                         start=True, stop=True)
                        done += pn
                        while gidx < len(groups) and \
                                groups[gidx][0] + groups[gidx][1] <= done:
                            m0, mn, r0 = groups[gidx]
                            nc.scalar.activation(r(E[:, m0:m0 + mn]),
                                                 ring[:, r0:r0 + mn], AF.Exp,
                                                 accum_out=rs[:, gidx:gidx + 1])
                            gidx += 1
                        if sb >= 0 and i < len(CH_2304):
                            bc0, bcn = CH_2304[i]
                            emit_bacc(sb, bc0, bcn)
                    for gi in range(len(groups), 4):
                        nc.vector.memset(rs[:, gi:gi + 1], 0.0)
                    rowsum = smalls.tile([C2, 1], F32, tag="rowsum",
                                         name=f"rowsum_{s}")
                    r2 = smalls.tile([C2, 1], F32, tag="r2", name=f"r2_{s}",
                                     bufs=4)
                    nc.vector.tensor_reduce(rowsum, rs, axis=mybir.AxisListType.X,
                                            op=mybir.AluOpType.add)
                    nc.vector.reciprocal(r2, rowsum)
                    r2s[s] = r2
                    if sb >= 0:
                        emit_csum(sb)
                    return E

                def emit_B_tail(sb):
                    wright = Wc[:, 128 * sb + 64:128 * sb + 128]
                    nc.vector.tensor_scalar_mul(r(wright), wright, r2s[sb])
                    for c0, cn in CH_2304:
                        emit_bacc(sb, c0, cn)
                    emit_csum(sb)

                Es = {}
                Es = {}

                for s in range(NSTRIP):
                    Es[s] = emit_A(s, phase, s - 2)
                    phase = (phase + len(CH_2304)) % 3
                for s in (NSTRIP - 2, NSTRIP - 1):
                    emit_B_tail(s)

                # drain P12 (both engines in parallel)
                nc.vector.tensor_copy(r(P12sb[0:C, :]), P12[0:C, :])
                nc.scalar.copy(r(P12sb[C:C2, :]), P12[C:C2, :])

            # ---- phase 3: 512-col pieces, one PSUM bank per role ----
            with tc.tile_pool(name="ph3p", bufs=1, space="PSUM") as ph3:
                for q, (p0, pn) in enumerate(CH_2304):
                    sl = slice(p0, p0 + pn)

                    cs = ph3.tile([1, pn], F32, tag="cs", name=f"cs_{q}",
                                  padded_shape=[1, 512])
                    half = HW // 2
                    if p0 + pn <= half:
                        nc.tensor.matmul(cs, r(ones128),
                                         r(csum_a[:, p0:p0 + pn]),
                                         start=True, stop=True)
                    elif p0 >= half:
                        nc.tensor.matmul(cs, r(ones128),
                                         r(csum_b[:, p0 - half:p0 - half + pn]),
                                         start=True, stop=True)
                    else:
                        ca = half - p0
                        nc.tensor.matmul(cs[:, 0:ca], r(ones128),
                                         r(csum_a[:, p0:half]),
                                         start=True, stop=True)
                        nc.tensor.matmul(cs[:, ca:pn], r(ones128),
                                         r(csum_b[:, 0:p0 + pn - half]),
                                         start=True, stop=True)
                    r1 = ph3sb.tile([1, pn], F32, tag="r1", name=f"r1_{q}",
                                    padded_shape=[1, 512])
                    nc.vector.reciprocal(r1, cs)

                    g1p = ph3.tile([1, pn], F32, tag="g1p", name=f"g1p_{q}",
                                   padded_shape=[1, 512])
                    nc.tensor.matmul(g1p, r(vlr[0:C]), r(P12sb[0:C, sl]),
                                     start=True, stop=True)
                    g2p = ph3.tile([1, pn], F32, tag="g2p", name=f"g2p_{q}",
                                   padded_shape=[1, 512])
                    nc.tensor.matmul(g2p, r(vlr[C:C2]), r(P12sb[C:C2, sl]),
                                     start=True, stop=True)

                    g1pre = ph3sb.tile([1, pn], F32, tag="g1pre",
                                       name=f"g1pre_{q}", padded_shape=[1, 512])
                    nc.vector.tensor_mul(g1pre, g1p, r1)
                    g1 = ph3sb.tile([1, pn], F32, tag="g1", name=f"g1_{q}",
                                    padded_shape=[1, 512])
                    nc.scalar.activation(g1, g1pre, AF.Sigmoid,
                                         bias=gb[0:1, 0:1], scale=1.0)
                    s1 = ph3sb.tile([1, pn], F32, tag="s1", name=f"s1_{q}",
                                    padded_shape=[1, 512])
                    nc.vector.tensor_mul(r(s1), g1, r1)
                    g2 = ph3sb.tile([1, pn], F32, tag="g2", name=f"g2_{q}",
                                    padded_shape=[1, 512])
                    nc.scalar.activation(r(g2), g2p, AF.Sigmoid,
                                         bias=gb[0:1, 1:2], scale=1.0)

                    S1 = ph3.tile([C, pn], F32, tag="S1", name=f"S1_{q}",
                                  padded_shape=[C, 512])
                    nc.tensor.matmul(S1, r(ones64), r(s1), start=True, stop=True)
                    S2 = ph3.tile([C, pn], F32, tag="S2", name=f"S2_{q}",
                                  padded_shape=[C, 512])
                    nc.tensor.matmul(S2, r(ones64), r(g2), start=True, stop=True)
                    t1 = ph3sb.tile([C, pn], F32, tag="t1", name=f"t1_{q}",
                                    padded_shape=[C, 512])
                    nc.vector.tensor_mul(r(t1), P12sb[0:C, sl], S1)
                    t2 = ph3sb.tile([C, pn], F32, tag="t2", name=f"t2_{q}",
                                    padded_shape=[C, 512])
                    nc.vector.tensor_mul(r(t2), P12sb[C:C2, sl], S2)

                    OL = ph3.tile([C, pn], F32, tag="OL", name=f"OL_{q}",
                                  padded_shape=[C, 512])
                    nc.tensor.matmul(OL, r(wloLT), r(XL[:, sl]),
                                     start=True, stop=False)
                    nc.tensor.matmul(OL, r(id64b[0:C]), r(t1),
                                     start=False, stop=True)
                    nc.scalar.activation(outLR[0:C, sl], OL, AF.Identity,
                                         bias=bLo, scale=1.0)
                    OR_ = ph3.tile([C, pn], F32, tag="OR", name=f"OR_{q}",
                                   padded_shape=[C, 512])
                    nc.tensor.matmul(OR_, r(wroLT), r(XR[:, sl]),
                                     start=True, stop=False)
                    nc.tensor.matmul(OR_, r(id64b[0:C]), r(t2),
                                     start=False, stop=True)
                    nc.scalar.activation(outLR[C:C2, sl], OR_, AF.Identity,
                                         bias=bRo, scale=1.0)
                    nc.sync.dma_start(out=out_l_d[:, sl], in_=outLR[0:C, sl])
                    nc.sync.dma_start(out=out_r_d[:, sl], in_=outLR[C:C2, sl])

    nc.compile()
    return nc


_NC_CACHE = {}


def _get_nc():
    if "nc" not in _NC_CACHE:
        _NC_CACHE["nc"] = build_nc()
    return _NC_CACHE["nc"]


def _prep_shared(concaL_w, concaL_b, concaR_w, concaR_b,
                 gateL_w, gateL_b, gateR_w, gateR_b,
                 concaLo_w, concaLo_b, concaRo_w, concaRo_b):
    f = np.float32
    wloR = np.asarray(concaLo_w)[:, C:].astype(np.float64)
    wroR = np.asarray(concaRo_w)[:, C:].astype(np.float64)
    vL = np.linalg.solve(wloR.T, np.asarray(gateL_w).astype(np.float64).reshape(C))
    vR = np.linalg.solve(wroR.T, np.asarray(gateR_w).astype(np.float64).reshape(C))
    vlr = np.concatenate([vL, vR]).reshape(C2, 1)
    return {
        "wlT": np.ascontiguousarray(np.asarray(concaL_w).T, dtype=f),
        "wrT": np.ascontiguousarray(np.asarray(concaR_w).T, dtype=f),
        "wloRT": np.ascontiguousarray(wloR.T, dtype=f),
        "wroRT": np.ascontiguousarray(wroR.T, dtype=f),
        "wloLT": np.ascontiguousarray(np.asarray(concaLo_w)[:, :C].T, dtype=f),
        "wroLT": np.ascontiguousarray(np.asarray(concaRo_w)[:, :C].T, dtype=f),
        "vlr": np.ascontiguousarray(vlr, dtype=f),
        "bL": np.ascontiguousarray(np.asarray(concaL_b).reshape(C, 1), dtype=f),
        "bR": np.ascontiguousarray(np.asarray(concaR_b).reshape(C, 1), dtype=f),
        "bLo": np.ascontiguousarray(np.asarray(concaLo_b).reshape(C, 1), dtype=f),
        "bRo": np.ascontiguousarray(np.asarray(concaRo_b).reshape(C, 1), dtype=f),
        "gb": np.array([[np.asarray(gateL_b).reshape(()),
                         np.asarray(gateR_b).reshape(())]], dtype=f),
    }


def kernel(xlh, xll, xrh, xrl,
           concaL_w, concaL_b, concaR_w, concaR_b,
           gateL_w, gateL_b, gateR_w, gateR_b,
           concaLo_w, concaLo_b, concaRo_w, concaRo_b,
           _return_results=False):
    nc = _get_nc()
    shared = _prep_shared(concaL_w, concaL_b, concaR_w, concaR_b,
                          gateL_w, gateL_b, gateR_w, gateR_b,
                          concaLo_w, concaLo_b, concaRo_w, concaRo_b)
    xlh = np.asarray(xlh, dtype=np.float32)
    xll = np.asarray(xll, dtype=np.float32)
    xrh = np.asarray(xrh, dtype=np.float32)
    xrl = np.asarray(xrl, dtype=np.float32)

    in_maps = []
    for c in range(B):
        x2l = np.concatenate([xlh[c].reshape(C, HW), xll[c].reshape(C, HW)], axis=0)
        x2r = np.concatenate([xrh[c].reshape(C, HW), xrl[c].reshape(C, HW)], axis=0)
        m = dict(shared)
        m["x2l"] = np.ascontiguousarray(x2l)
        m["x2r"] = np.ascontiguousarray(x2r)
        in_maps.append(m)

    res = run_bass_kernel_spmd(nc, in_maps, list(range(B)))
    out_L = np.stack([res.results[c]["out_l"].reshape(C, H, W) for c in range(B)])
    out_R = np.stack([res.results[c]["out_r"].reshape(C, H, W) for c in range(B)])
    if _return_results:
        return (out_L, out_R), res
    return (out_L, out_R)


# revision 27
# speedup vs baseline: 6851.3497x; 6709.8544x over previous
"""CoAttenBlock Trainium2 kernel.

Full inputs in, full outputs out. Shards batch (B=8) across 8 NeuronCores,
one sample per core (pure data parallel, no collectives).

Per-core math (C=64, HW=2304, strips of 128 along the left position axis n):
  XL = WL @ [xlh;xll] + bL                      [64, 2304]
  XR = WR @ [xrh;xrl] + bR
  per strip s: aff_s = XL[:,s].T @ XR           [128, 2304]
               E_s   = exp(aff_s), rowsum via activation accum_out
               r2    = 1/rowsum  (folded into the strip's YRT weight columns)
               P12  += [YLT_s | YRT_s*r2].T @ E_s   (PSUM-resident [128, 2304])
  where YLT = (WLo_r @ XL).T strips, YRT = (WRo_r @ XR).T strips, so that
  P1 = WLo_r @ (XL @ E) and P2 = WRo_r @ (XR @ diag(r2) @ E).
  Gate pre-acts are recovered as vL.T @ P1 with vL = solve(WLo_r.T, gwL.T)
  (host-side 64x64 solve; inputs are deterministic, conditioning ~5e2).
  colsum = ones.T @ (sum of E_s)  (two SBUF accumulators: DVE + GPSIMD chains)
  s1 = sigmoid(g1pre * r1 + gb1) * r1,  r1 = 1/colsum ; s2 = sigmoid(g2pre+gb2)
  out_L = WLo_l @ XL + P1 * s1[m] + bLo ; out_R = WRo_l @ XR + P2 * s2[m] + bRo

float32r (single-pass PE mode) is used for all large matmuls; producers of
f32r-matmul inputs write with a f32r-typed output AP so the engine rounds on
write (BIR verifier requirement). Convs and YLT/YRT stay higher precision.
"""

import os
import sys

import numpy as np

if os.path.isdir("/opt/trn_rl_repo") and "/opt/trn_rl_repo" not in sys.path:
    sys.path.insert(0, "/opt/trn_rl_repo")

import concourse.bass as bass
import concourse.tile as tile
from concourse import bacc, mybir
from concourse.bass_utils import run_bass_kernel_spmd

B, C, H, W = 8, 64, 48, 48
HW = H * W            # 2304
C2 = 2 * C            # 128
NSTRIP = HW // 128    # 18
F32 = mybir.dt.float32
F32R = mybir.dt.float32r
AF = mybir.ActivationFunctionType


def chunks(total, step=512):
    out = []
    c0 = 0
    while c0 < total:
        out.append((c0, min(step, total - c0)))
        c0 += step
    return out


CH_2304 = chunks(2304)            # 4x512 + 256


def r(ap):
    return ap.bitcast(F32R)


def build_nc():
    nc = bacc.Bacc("TRN2", target_bir_lowering=False, debug=False)

    x2l_d = nc.dram_tensor("x2l", [C2, HW], F32, kind="ExternalInput").ap()
    x2r_d = nc.dram_tensor("x2r", [C2, HW], F32, kind="ExternalInput").ap()
    wlT_d = nc.dram_tensor("wlT", [C2, C], F32, kind="ExternalInput").ap()
    wrT_d = nc.dram_tensor("wrT", [C2, C], F32, kind="ExternalInput").ap()
    wloRT_d = nc.dram_tensor("wloRT", [C, C], F32, kind="ExternalInput").ap()
    wroRT_d = nc.dram_tensor("wroRT", [C, C], F32, kind="ExternalInput").ap()
    wloLT_d = nc.dram_tensor("wloLT", [C, C], F32, kind="ExternalInput").ap()
    wroLT_d = nc.dram_tensor("wroLT", [C, C], F32, kind="ExternalInput").ap()
    vlr_d = nc.dram_tensor("vlr", [C2, 1], F32, kind="ExternalInput").ap()
    bL_d = nc.dram_tensor("bL", [C, 1], F32, kind="ExternalInput").ap()
    bR_d = nc.dram_tensor("bR", [C, 1], F32, kind="ExternalInput").ap()
    bLo_d = nc.dram_tensor("bLo", [C, 1], F32, kind="ExternalInput").ap()
    bRo_d = nc.dram_tensor("bRo", [C, 1], F32, kind="ExternalInput").ap()
    gb_d = nc.dram_tensor("gb", [1, 2], F32, kind="ExternalInput").ap()
    # identity stacked twice: rows 0:64 and 64:128 both hold I_64, so id-adds
    # can source either half of a [128, ...] tile at matching base partition
    id2_np = np.vstack([np.eye(C, dtype=np.float32), np.eye(C, dtype=np.float32)])
    id64b_d = nc.inline_tensor(id2_np, "id64b").ap()
    # selector for the merged S12 broadcast: out rows 0:64 <- s1, 64:128 <- g2
    sel_np = np.zeros((2, C2), dtype=np.float32)
    sel_np[0, 0:C] = 1.0
    sel_np[1, C:C2] = 1.0
    sel12_d = nc.inline_tensor(sel_np, "sel12").ap()

    out_l_d = nc.dram_tensor("out_l", [C, HW], F32, kind="ExternalOutput").ap()
    out_r_d = nc.dram_tensor("out_r", [C, HW], F32, kind="ExternalOutput").ap()

    with tile.TileContext(nc) as tc:
        import contextlib

        with contextlib.ExitStack() as outer:
            consts = outer.enter_context(tc.tile_pool(name="consts", bufs=1))
            big = outer.enter_context(tc.tile_pool(name="big", bufs=1))
            epool = outer.enter_context(tc.tile_pool(name="epool", bufs=4))
            smalls = outer.enter_context(tc.tile_pool(name="smalls", bufs=3))
            ph3sb = outer.enter_context(tc.tile_pool(name="ph3sb", bufs=2))

            # ---- constants / weights to SBUF ----
            wlT = consts.tile([C2, C], F32)
            wrT = consts.tile([C2, C], F32)
            wloRT_raw = consts.tile([C, C], F32)
            wroRT_raw = consts.tile([C, C], F32)
            wloLT_raw = consts.tile([C, C], F32)
            wroLT_raw = consts.tile([C, C], F32)
            vlr_raw = consts.tile([C2, 1], F32)
            id64b_raw = consts.tile([C2, C], F32)
            sel12_raw = consts.tile([2, C2], F32)
            ones128_raw = consts.tile([C2, 1], F32)
            ones64_raw = consts.tile([1, C], F32)
            wloRT = consts.tile([C, C], F32)
            wroRT = consts.tile([C, C], F32)
            wloLT = consts.tile([C, C], F32)
            wroLT = consts.tile([C, C], F32)
            vlr = consts.tile([C2, 1], F32)
            id64b = consts.tile([C2, C], F32)
            sel12 = consts.tile([2, C2], F32)
            ones128 = consts.tile([C2, 1], F32)
            ones64 = consts.tile([1, C], F32)
            bL = consts.tile([C, 1], F32)
            bR = consts.tile([C, 1], F32)
            bLo = consts.tile([C, 1], F32)
            bRo = consts.tile([C, 1], F32)
            gb = consts.tile([1, 2], F32)
            for dst, src in [
                (r(wlT), r(wlT_d)), (r(wrT), r(wrT_d)),
                (wloRT_raw, wloRT_d), (wroRT_raw, wroRT_d),
                (wloLT_raw, wloLT_d), (wroLT_raw, wroLT_d),
                (vlr_raw, vlr_d), (id64b_raw, id64b_d), (sel12_raw, sel12_d),
                (bL, bL_d), (bR, bR_d), (bLo, bLo_d), (bRo, bRo_d),
                (gb, gb_d),
            ]:
                nc.sync.dma_start(out=dst, in_=src)
            nc.vector.memset(ones128_raw, 1.0)
            nc.vector.memset(ones64_raw, 1.0)
            for dst, srcc in [(ones128, ones128_raw), (ones64, ones64_raw),
                              (wloRT, wloRT_raw), (wroRT, wroRT_raw),
                              (wloLT, wloLT_raw), (wroLT, wroLT_raw),
                              (vlr, vlr_raw), (id64b, id64b_raw),
                              (sel12, sel12_raw)]:
                nc.scalar.copy(r(dst), srcc)

            # ---- big SBUF tensors ----
            x2l = big.tile([C2, HW], F32)
            x2r = big.tile([C2, HW], F32)
            XL = big.tile([C, HW], F32)
            XR = big.tile([C, HW], F32)
            Wc = big.tile([C2, HW], F32)       # 18 strips of [YLT | YRT]
            csum_a = big.tile([C2, HW // 2], F32)  # DVE accumulates m[0:1152]
            csum_b = big.tile([C2, HW // 2], F32)  # Pool accumulates m[1152:]
            P12sb = big.tile([C2, HW], F32)    # drained P1 (0:64) / P2 (64:128)
            outLR = big.tile([C2, HW], F32)

            for c0, cn in CH_2304:
                nc.sync.dma_start(out=r(x2l[:, c0:c0 + cn]),
                                  in_=r(x2l_d[:, c0:c0 + cn]))
                nc.sync.dma_start(out=r(x2r[:, c0:c0 + cn]),
                                  in_=r(x2r_d[:, c0:c0 + cn]))

            with contextlib.ExitStack() as ph2_psum:
                p12p = ph2_psum.enter_context(
                    tc.tile_pool(name="p12p", bufs=1, space="PSUM"))
                affp = ph2_psum.enter_context(
                    tc.tile_pool(name="affp", bufs=1, space="PSUM"))
                P12 = p12p.tile([C2, HW], F32)  # 5 banks, lives all of phase 1+2
                ring = affp.tile([C2, 1536], F32, tag="ring", name="aff_ring")

                # ---- phase 1: convs (full fp32) + YLT/YRT build ----
                # After conv chunk j, emit the YLT/YRT strips of chunk j-1
                # (their XL/XR columns are copied by then); P12 is scratch.
                def emit_y(t):
                    ysl = slice(64 * t, 64 * t + 64)
                    nc.tensor.matmul(P12[:, ysl],
                                     r(XL[:, 128 * t:128 * t + 128]),
                                     r(wloRT), start=True, stop=True)
                    nc.vector.tensor_copy(r(Wc[:, 128 * t:128 * t + 64]),
                                          P12[:, ysl])
                    ysr = slice(64 * (NSTRIP + t), 64 * (NSTRIP + t) + 64)
                    nc.tensor.matmul(P12[:, ysr],
                                     r(XR[:, 128 * t:128 * t + 128]),
                                     r(wroRT), start=True, stop=True)
                    nc.vector.tensor_copy(r(Wc[:, 128 * t + 64:128 * t + 128]),
                                          P12[:, ysr])

                for j, (c0, cn) in enumerate(CH_2304):
                    nc.tensor.matmul(P12[0:C, c0:c0 + cn], r(wlT),
                                     r(x2l[:, c0:c0 + cn]), start=True, stop=True)
                    nc.scalar.activation(r(XL[:, c0:c0 + cn]),
                                         P12[0:C, c0:c0 + cn],
                                         AF.Identity, bias=bL, scale=1.0)
                    rsl = (j % 3) * 512
                    nc.tensor.matmul(ring[0:C, rsl:rsl + cn], r(wrT),
                                     r(x2r[:, c0:c0 + cn]), start=True, stop=True)
                    nc.vector.tensor_scalar_add(r(XR[:, c0:c0 + cn]),
                                                ring[0:C, rsl:rsl + cn], bR)
                    if j > 0:
                        for t in range(4 * (j - 1), 4 * j):
                            emit_y(t)
                for t in range(4 * (len(CH_2304) - 1), NSTRIP):
                    emit_y(t)

                # ---- phase 2: strip loop over a 3-slot aff ring ----
                # A_s = aff matmuls + merged exps + rowsum/recip for strip s.
                # B_s = Wc scale + bacc matmuls + colsum accumulate for s.
                # B lags A by 2 strips so PE always has bacc work to fill exp
                # waits; the YLT/YRT -> Wc build is emitted during the lag.
                phase = 0
                r2s = {}

                def emit_bacc(sb, c0, cn):
                    nc.tensor.matmul(P12[:, c0:c0 + cn],
                                     r(Wc[:, 128 * sb:128 * sb + 128]),
                                     r(Es[sb][:, c0:c0 + cn]),
                                     start=(sb == 0), stop=(sb == NSTRIP - 1))

                def emit_csum(sb):
                    E = Es[sb]
                    half = HW // 2
                    if sb == 0:
                        nc.vector.tensor_copy(r(csum_a), E[:, 0:half])
                        nc.gpsimd.tensor_copy(r(csum_b), E[:, half:HW])
                    else:
                        nc.vector.tensor_add(r(csum_a), csum_a, E[:, 0:half])
                        nc.gpsimd.tensor_add(r(csum_b), csum_b, E[:, half:HW])

                def emit_A(s, phase, sb):
                    # aff+exp for strip s, with strip sb's bacc matmuls
                    # interleaved between the aff pieces (PE is in-order; this
                    # keeps ACT fed with the next exp as early as possible).
                    if sb >= 0:
                        wright = Wc[:, 128 * sb + 64:128 * sb + 128]
                        nc.vector.tensor_scalar_mul(r(wright), wright, r2s[sb])
                    E = epool.tile([C2, HW], F32, tag="e", name=f"E_{s}")
                    rs = smalls.tile([C2, 4], F32, tag="rs", name=f"rs_{s}")
                    lhs_aff = r(XL[:, 128 * s:128 * s + 128])
                    pieces = [(p0, pn, (phase + i) % 3)
                              for i, (p0, pn) in enumerate(CH_2304)]
                    groups = []
                    for p0, pn, sl in pieces:
                        if groups and groups[-1][2] + groups[-1][1] == sl * 512 \
                                and groups[-1][1] + pn <= 1536:
                            groups[-1][1] += pn
                        else:
                            groups.append([p0, pn, sl * 512])
                    gidx = 0
                    done = 0
                    for i, (p0, pn, sl) in enumerate(pieces):
                        nc.tensor.matmul(ring[:, sl * 512:sl * 512 + pn],
                                         lhs_aff, r(XR[:, p0:p0 + pn]),
                                         start=True, stop=True)
                        done += pn
                        while gidx < len(groups) and \
                                groups[gidx][0] + groups[gidx][1] <= done:
                            m0, mn, r0 = groups[gidx]
                            nc.scalar.activation(r(E[:, m0:m0 + mn]),
                                                 ring[:, r0:r0 + mn], AF.Exp,
                                                 accum_out=rs[:, gidx:gidx + 1])
                            gidx += 1
                        if sb >= 0 and i < len(CH_2304):
                            bc0, bcn = CH_2304[i]
                            emit_bacc(sb, bc0, bcn)
                    rowsum = smalls.tile([C2, 1], F32, tag="rowsum",
                                         name=f"rowsum_{s}")
                    r2 = smalls.tile([C2, 1], F32, tag="r2", name=f"r2_{s}",
                                     bufs=4)
                    nc.vector.tensor_reduce(rowsum, rs[:, 0:len(groups)],
                                            axis=mybir.AxisListType.X,
                                            op=mybir.AluOpType.add)
                    nc.vector.reciprocal(r2, rowsum)
                    r2s[s] = r2
                    if sb >= 0:
                        emit_csum(sb)
                    return E

                def emit_B_tail(sb):
                    wright = Wc[:, 128 * sb + 64:128 * sb + 128]
                    nc.vector.tensor_scalar_mul(r(wright), wright, r2s[sb])
                    for c0, cn in CH_2304:
                        emit_bacc(sb, c0, cn)
                    emit_csum(sb)

                Es = {}
                Es = {}

                for s in range(NSTRIP):
                    Es[s] = emit_A(s, phase, s - 2)
                    phase = (phase + len(CH_2304)) % 3
                for s in (NSTRIP - 2, NSTRIP - 1):
                    emit_B_tail(s)

                # drain P12 (both engines in parallel)
                nc.vector.tensor_copy(r(P12sb[0:C, :]), P12[0:C, :])
                nc.scalar.copy(r(P12sb[C:C2, :]), P12[C:C2, :])

            # ---- phase 3: 512-col pieces, one PSUM bank per role ----
            with tc.tile_pool(name="ph3p", bufs=1, space="PSUM") as ph3:
                for q, (p0, pn) in enumerate(CH_2304):
                    sl = slice(p0, p0 + pn)

                    cs = ph3.tile([1, pn], F32, tag="cs", name=f"cs_{q}",
                                  padded_shape=[1, 512])
                    half = HW // 2
                    if p0 + pn <= half:
                        nc.tensor.matmul(cs, r(ones128),
                                         r(csum_a[:, p0:p0 + pn]),
                                         start=True, stop=True)
                    elif p0 >= half:
                        nc.tensor.matmul(cs, r(ones128),
                                         r(csum_b[:, p0 - half:p0 - half + pn]),
                                         start=True, stop=True)
                    else:
                        ca = half - p0
                        nc.tensor.matmul(cs[:, 0:ca], r(ones128),
                                         r(csum_a[:, p0:half]),
                                         start=True, stop=True)
                        nc.tensor.matmul(cs[:, ca:pn], r(ones128),
                                         r(csum_b[:, 0:p0 + pn - half]),
                                         start=True, stop=True)
                    r1 = ph3sb.tile([1, pn], F32, tag="r1", name=f"r1_{q}",
                                    padded_shape=[1, 512])
                    nc.vector.reciprocal(r1, cs)

                    g1p = ph3.tile([1, pn], F32, tag="g1p", name=f"g1p_{q}",
                                   padded_shape=[1, 512])
                    nc.tensor.matmul(g1p, r(vlr[0:C]), r(P12sb[0:C, sl]),
                                     start=True, stop=True)
                    g2p = ph3.tile([1, pn], F32, tag="g2p", name=f"g2p_{q}",
                                   padded_shape=[1, 512])
                    nc.tensor.matmul(g2p, r(vlr[C:C2]), r(P12sb[C:C2, sl]),
                                     start=True, stop=True)

                    g1pre = ph3sb.tile([1, pn], F32, tag="g1pre",
                                       name=f"g1pre_{q}", padded_shape=[1, 512])
                    nc.vector.tensor_mul(g1pre, g1p, r1)
                    g1 = ph3sb.tile([1, pn], F32, tag="g1", name=f"g1_{q}",
                                    padded_shape=[1, 512])
                    nc.scalar.activation(g1, g1pre, AF.Sigmoid,
                                         bias=gb[0:1, 0:1], scale=1.0)
                    s1 = ph3sb.tile([1, pn], F32, tag="s1", name=f"s1_{q}",
                                    padded_shape=[1, 512])
                    nc.vector.tensor_mul(r(s1), g1, r1)
                    g2 = ph3sb.tile([1, pn], F32, tag="g2", name=f"g2_{q}",
                                    padded_shape=[1, 512])
                    nc.scalar.activation(r(g2), g2p, AF.Sigmoid,
                                         bias=gb[0:1, 1:2], scale=1.0)

                    S1 = ph3.tile([C, pn], F32, tag="S1", name=f"S1_{q}",
                                  padded_shape=[C, 512])
                    nc.tensor.matmul(S1, r(ones64), r(s1), start=True, stop=True)
                    S2 = ph3.tile([C, pn], F32, tag="S2", name=f"S2_{q}",
                                  padded_shape=[C, 512])
                    nc.tensor.matmul(S2, r(ones64), r(g2), start=True, stop=True)
                    t1 = ph3sb.tile([C, pn], F32, tag="t1", name=f"t1_{q}",
                                    padded_shape=[C, 512])
                    nc.vector.tensor_mul(r(t1), P12sb[0:C, sl], S1)
                    t2 = ph3sb.tile([C, pn], F32, tag="t2", name=f"t2_{q}",
                                    padded_shape=[C, 512])
                    nc.vector.tensor_mul(r(t2), P12sb[C:C2, sl], S2)

                    OL = ph3.tile([C, pn], F32, tag="OL", name=f"OL_{q}",
                                  padded_shape=[C, 512])
                    nc.tensor.matmul(OL, r(wloLT), r(XL[:, sl]),
                                     start=True, stop=False)
                    nc.tensor.matmul(OL, r(id64b[0:C]), r(t1),
                                     start=False, stop=True)
                    nc.scalar.activation(outLR[0:C, sl], OL, AF.Identity,
                                         bias=bLo, scale=1.0)
                    OR_ = ph3.tile([C, pn], F32, tag="OR", name=f"OR_{q}",
                                   padded_shape=[C, 512])
                    nc.tensor.matmul(OR_, r(wroLT), r(XR[:, sl]),
                                     start=True, stop=False)
                    nc.tensor.matmul(OR_, r(id64b[0:C]), r(t2),
                                     start=False, stop=True)
                    nc.scalar.activation(outLR[C:C2, sl], OR_, AF.Identity,
                                         bias=bRo, scale=1.0)
                    nc.sync.dma_start(out=out_l_d[:, sl], in_=outLR[0:C, sl])
                    nc.sync.dma_start(out=out_r_d[:, sl], in_=outLR[C:C2, sl])

    nc.compile()
    return nc


_NC_CACHE = {}


def _get_nc():
    if "nc" not in _NC_CACHE:
        _NC_CACHE["nc"] = build_nc()
    return _NC_CACHE["nc"]


def _prep_shared(concaL_w, concaL_b, concaR_w, concaR_b,
                 gateL_w, gateL_b, gateR_w, gateR_b,
                 concaLo_w, concaLo_b, concaRo_w, concaRo_b):
    f = np.float32
    wloR = np.asarray(concaLo_w)[:, C:].astype(np.float64)
    wroR = np.asarray(concaRo_w)[:, C:].astype(np.float64)
    vL = np.linalg.solve(wloR.T, np.asarray(gateL_w).astype(np.float64).reshape(C))
    vR = np.linalg.solve(wroR.T, np.asarray(gateR_w).astype(np.float64).reshape(C))
    vlr = np.concatenate([vL, vR]).reshape(C2, 1)
    return {
        "wlT": np.ascontiguousarray(np.asarray(concaL_w).T, dtype=f),
        "wrT": np.ascontiguousarray(np.asarray(concaR_w).T, dtype=f),
        "wloRT": np.ascontiguousarray(wloR.T, dtype=f),
        "wroRT": np.ascontiguousarray(wroR.T, dtype=f),
        "wloLT": np.ascontiguousarray(np.asarray(concaLo_w)[:, :C].T, dtype=f),
        "wroLT": np.ascontiguousarray(np.asarray(concaRo_w)[:, :C].T, dtype=f),
        "vlr": np.ascontiguousarray(vlr, dtype=f),
        "bL": np.ascontiguousarray(np.asarray(concaL_b).reshape(C, 1), dtype=f),
        "bR": np.ascontiguousarray(np.asarray(concaR_b).reshape(C, 1), dtype=f),
        "bLo": np.ascontiguousarray(np.asarray(concaLo_b).reshape(C, 1), dtype=f),
        "bRo": np.ascontiguousarray(np.asarray(concaRo_b).reshape(C, 1), dtype=f),
        "gb": np.array([[np.asarray(gateL_b).reshape(()),
                         np.asarray(gateR_b).reshape(())]], dtype=f),
    }


def kernel(xlh, xll, xrh, xrl,
           concaL_w, concaL_b, concaR_w, concaR_b,
           gateL_w, gateL_b, gateR_w, gateR_b,
           concaLo_w, concaLo_b, concaRo_w, concaRo_b,
           _return_results=False):
    nc = _get_nc()
    shared = _prep_shared(concaL_w, concaL_b, concaR_w, concaR_b,
                          gateL_w, gateL_b, gateR_w, gateR_b,
                          concaLo_w, concaLo_b, concaRo_w, concaRo_b)
    xlh = np.asarray(xlh, dtype=np.float32)
    xll = np.asarray(xll, dtype=np.float32)
    xrh = np.asarray(xrh, dtype=np.float32)
    xrl = np.asarray(xrl, dtype=np.float32)

    in_maps = []
    for c in range(B):
        x2l = np.concatenate([xlh[c].reshape(C, HW), xll[c].reshape(C, HW)], axis=0)
        x2r = np.concatenate([xrh[c].reshape(C, HW), xrl[c].reshape(C, HW)], axis=0)
        m = dict(shared)
        m["x2l"] = np.ascontiguousarray(x2l)
        m["x2r"] = np.ascontiguousarray(x2r)
        in_maps.append(m)

    res = run_bass_kernel_spmd(nc, in_maps, list(range(B)))
    out_L = np.stack([res.results[c]["out_l"].reshape(C, H, W) for c in range(B)])
    out_R = np.stack([res.results[c]["out_r"].reshape(C, H, W) for c in range(B)])
    if _return_results:
        return (out_L, out_R), res
    return (out_L, out_R)
